# revision 2
# baseline (speedup 1.0000x reference)
"""Trainium2 Bass kernel for BondEncoding2D (Graphormer-style bond encoding).

Computes, for a 512x512 node-pair grid:
  phi_spd[h,i,j]  = spd_table[spatial_pos[i,j], h]
  phi_edge[h,i,j] = (sum_d edge_table[edge_input[i,j,d]] @ W[d])[h] / max(spatial_pos[i,j],1)

Sharding: rows of the grid across 8 NeuronCores (64 rows / 32768 pairs each);
tables and weights replicated (per the sharding hint).

Per-core strategy (v2):
  * Host precomputes M[d] = edge_table @ W[d]; the edge term is then
    edge_sum[pair,:] = sum_d M[d, e_d(pair), :] — a 512-wide multi-hot
    (16 ones) contraction per pair.
  * The 512-wide multi-hot over c=(d,bond) is built exactly as
    relu(1 - (e_d - b)^2): a feature matmul (features [1, e_d, e_d^2-split]
    sent from host, exact in bf16) computes the integer argument on the PE
    (two concurrent 64-row bands), and the relu runs natively split across
    DVE and ACT producing exact 0/1 bf16.
  * Main matmuls: M (single bf16, chunk-major [128c' x 32k] per chunk) is
    the STATIONARY operand (32-col ldweights, cheap); the one-hot chunk is
    the MOVING operand (N=512/matmul), accumulating the 4 c-chunks into a
    [32, 512] PSUM strip. Tile t lands on column strip t%4, so 4 tiles
    share one [128, 512] PSUM bank and a single fused DVE epilogue
    multiplies by 1/denom (host-replicated) and converts to bf16.
  * phi_spd is a single exact-f32 GPSIMD ap_gather (runs concurrently with
    the PE pipeline).
  * Host reassembles the (32,512,512) outputs from the device layouts.
"""

import numpy as np
import ml_dtypes

import concourse.bass as bass
import concourse.bacc as bacc
import concourse.mybir as mybir
import concourse.tile as tile
from concourse.bass_utils import run_bass_kernel_spmd

N = 512          # atoms
D = 16           # max_dist
H = 32           # heads
NS = 64          # spatial values
NCORES = 8
RC = N // NCORES          # rows per core (64)
PC = RC * N               # pairs per core (32768)

TILES = 64                # PE tiles per core
TP = 512                  # pairs per tile (= one grid row)
NF = 64                   # feature rows (1 + 16*3 used, padded)
GT = 4                    # tiles per psum group (4 col strips)
NGRP = TILES // GT        # psum groups (16)
STAGE_G = 4               # groups per output DMA (16 tiles)
DVE_COLS = 840            # relu columns (of 1024 in ctA) done on DVE

# spd gather side
NBLK = 8                  # pair blocks (one per Q7 core)
SCALLS = 8                # gather calls
SJT = PC // NBLK // SCALLS  # pairs per block per call (512)

BF16 = mybir.dt.bfloat16
F32 = mybir.dt.float32
I16 = mybir.dt.int16

_cached = {}


def _build_nc(bench_reps=None, parts=("spd", "edge"), INTERLEAVE=True):
    nc = bacc.Bacc(None, target_bir_lowering=False)

    afeat = nc.dram_tensor("afeat", [128, 512], BF16, kind="ExternalInput")
    mstat = nc.dram_tensor("mstat", [128, 128], BF16, kind="ExternalInput")
    feat = nc.dram_tensor("feat", [NF, PC], BF16, kind="ExternalInput")
    tab_s = nc.dram_tensor("tab_s", [128, 4096 * 4], F32, kind="ExternalInput")
    sidx = nc.dram_tensor("sidx", [128, SCALLS * SJT // 32], I16,
                          kind="ExternalInput")
    rrep4 = nc.dram_tensor("rrep4", [128, PC // 4], F32, kind="ExternalInput")
    oedge = nc.dram_tensor("oedge", [128, PC // 4], BF16,
                           kind="ExternalOutput")
    ospd = nc.dram_tensor("ospd", [128, SCALLS * SJT * 2], F32,
                          kind="ExternalOutput")

    mult = mybir.AluOpType.mult
    RELU = mybir.ActivationFunctionType.Relu

    with tile.TileContext(nc) as tc:
        with (
            tc.tile_pool(name="consts", bufs=1) as cpool,
            tc.tile_pool(name="arg_a", bufs=2, space="PSUM") as agpool_a,
            tc.tile_pool(name="arg_b", bufs=1, space="PSUM") as agpool_b,
            tc.tile_pool(name="outp", bufs=2, space="PSUM") as oppool,
            tc.tile_pool(name="ct", bufs=3) as ctpool,
            tc.tile_pool(name="stage", bufs=2) as stpool,
            tc.tile_pool(name="spd", bufs=2) as spool,
        ):
            afeat_t = cpool.tile([128, 512], BF16)
            nc.sync.dma_start(afeat_t[:], afeat[:])
            mstat_t = cpool.tile([128, 128], BF16)
            nc.sync.dma_start(mstat_t[:], mstat[:])
            tabs_t = cpool.tile([128, 4096 * 4], F32)
            nc.sync.dma_start(tabs_t[:], tab_s[:])
            rrep_t = cpool.tile([128, PC // 4], F32)
            nc.sync.dma_start(rrep_t[:], rrep4[:])
            featrep = cpool.tile([128, PC], BF16)
            for half in range(2):
                nc.sync.dma_start(featrep[64 * half:64 * half + NF, :], feat[:])

            import contextlib
            loop_cm = (
                tc.For_i(0, bench_reps, 1) if bench_reps
                else contextlib.nullcontext()
            )
            with loop_cm:
                # ---- phi_edge PE pipeline with spd gathers interleaved
                def spd_call(s):
                    si = spool.tile([128, SJT // 32], I16, tag="si")
                    nc.sync.dma_start(
                        si[:], sidx[:, s * (SJT // 32):(s + 1) * (SJT // 32)]
                    )
                    gs = spool.tile([128, SJT * 2], F32, tag="gs")
                    nc.gpsimd.ap_gather(
                        gs[:].rearrange("p (i v) -> p i v", v=4),
                        tabs_t[:].rearrange("p (n v) -> p n v", v=4),
                        si[:], channels=128, num_elems=4096, d=4,
                        num_idxs=SJT // 2,
                    )
                    nc.sync.dma_start(
                        ospd[:, s * SJT * 2:(s + 1) * SJT * 2], gs[:]
                    )

                if "edge" not in parts or not INTERLEAVE:
                    for s in range(SCALLS if "spd" in parts else 0):
                        spd_call(s)
                import collections
                pend = collections.deque()
                SKEW = 2
                ntl = TILES if "edge" in parts else 0
                op = None
                st = None
                for tt in range(ntl + SKEW):
                    if tt < ntl:
                        t = tt
                        if ("spd" in parts and INTERLEAVE
                                and t % (TILES // SCALLS) == 0):
                            spd_call(t // (TILES // SCALLS))
                        # one-hot args: arg[c,pair] = 1-(e_d(c)-b(c))^2 on PE
                        args = []
                        for ab in range(2):   # two double-bank psum tiles
                            pool = agpool_a if ab == 0 else agpool_b
                            ag = pool.tile([128, 2 * TP], F32, tag=f"ag{ab}")
                            for qq in range(2):   # chunks q = 2*ab + qq
                                q = 2 * ab + qq
                                nc.tensor.matmul(
                                    ag[:, qq * TP:(qq + 1) * TP],
                                    afeat_t[64 * qq:64 * qq + 64,
                                            128 * q:128 * q + 128],
                                    featrep[64 * qq:64 * qq + 64,
                                            t * TP:(t + 1) * TP],
                                    start=True, stop=True,
                                    tile_position=(64 * qq, 0),
                                )
                            args.append(ag)
                        # relu -> exact one-hot bf16 (DVE most, ACT rest)
                        ctA = ctpool.tile([128, 2 * TP], BF16, tag="ctA")
                        nc.vector.tensor_relu(
                            ctA[:, :DVE_COLS], args[0][:, :DVE_COLS])
                        nc.scalar.activation(
                            ctA[:, DVE_COLS:], args[0][:, DVE_COLS:], RELU)
                        ctB = ctpool.tile([128, 2 * TP], BF16, tag="ctB")
                        nc.scalar.activation(ctB[:], args[1][:], RELU)
                        pend.append((t, ctA, ctB))
                    if tt < SKEW - 1 or not pend:
                        continue
                    if tt < ntl and len(pend) <= SKEW - 1:
                        continue
                    t, ctA, ctB = pend.popleft()
                    # main matmuls: M stationary (32 cols), one-hot moving.
                    # tile t -> column strip a=t%4 of the group psum tile.
                    a = t % GT
                    if a == 0:
                        op = oppool.tile([128, TP], F32, tag="op")
                    for q in range(4):
                        ct = ctA if q < 2 else ctB
                        qq = q % 2
                        nc.tensor.matmul(
                            op[32 * a:32 * a + 32, :],
                            mstat_t[:, 32 * q:32 * q + 32],
                            ct[:, qq * TP:(qq + 1) * TP],
                            start=(q == 0), stop=(q == 3),
                            tile_position=(0, 32 * a),
                        )
                    if a == GT - 1:
                        # fused epilogue for the 4-tile group: x (1/denom),
                        # f32 PSUM -> bf16 staging SBUF
                        s = t // GT
                        sl = s % STAGE_G
                        if sl == 0:
                            st = stpool.tile([128, TP * STAGE_G], BF16,
                                             tag="st")
                        nc.vector.tensor_tensor(
                            st[:, sl * TP:(sl + 1) * TP], op[:],
                            rrep_t[:, s * TP:(s + 1) * TP], mult)
                        if sl == STAGE_G - 1:
                            b = s // STAGE_G
                            nc.sync.dma_start(
                                oedge[:, b * TP * STAGE_G:
                                      (b + 1) * TP * STAGE_G],
                                st[:],
                            )
    nc.compile()
    return nc


def _host_prep(spatial_pos, edge_input, max_dist, spd_table, edge_table,
               edge_dis_weight):
    """Build per-core input maps (all numpy)."""
    md = int(max_dist)
    assert md == D
    W = edge_dis_weight.reshape(-1, H, H)[:md].astype(np.float64)
    M = edge_table.astype(np.float64) @ W          # (16, 32, 32)

    cp = np.arange(128)
    bb = (cp % 32).astype(np.float64)              # bond id per c'
    # feature-matmul weights: arg = (1-b^2)*1 + 2b*e_d - e2hi_d - e2lo_d
    afeat = np.zeros((128, 512), np.float64)
    for q in range(4):
        dsel = 4 * q + cp // 32                    # d(c') per column
        blk = np.zeros((64, 128))
        cc = 1.0 - bb * bb
        cchi = cc.astype(ml_dtypes.bfloat16).astype(np.float64)
        blk[0, :] = cchi
        blk[49, :] = cc - cchi
        blk[1 + dsel, cp] = 2.0 * bb
        blk[17 + dsel, cp] = -1.0
        blk[33 + dsel, cp] = -1.0
        afeat[64 * (q % 2):64 * (q % 2) + 64, 128 * q:128 * q + 128] = blk
    afeat = afeat.astype(ml_dtypes.bfloat16)

    # mstat[c', 32q+k] = bf16(M[4q + c'//32, c'%32, k])  (chunk-major)
    mstat = np.zeros((128, 128), ml_dtypes.bfloat16)
    for q in range(4):
        mstat[:, 32 * q:32 * q + 32] = M[4 * q + cp // 32, cp % 32, :]

    # spd pair-combined gather table: partition 16g+hh holds head cols
    # (2hh, 2hh+1) for both members of the pair-pair (sa, sb) = (c//64, c%64)
    hh = (np.arange(128) % 16)
    X = np.ascontiguousarray(
        spd_table[:, np.stack([2 * hh, 2 * hh + 1], 1)].transpose(1, 0, 2)
    ).astype(np.float32)                           # (128, 64, 2)
    T4 = np.empty((128, NS, NS, 4), np.float32)
    T4[:, :, :, 0:2] = X[:, :, None, :]
    T4[:, :, :, 2:4] = X[:, None, :, :]
    tab_s = T4.reshape(128, 4096 * 4)

    in_maps = []
    for c in range(NCORES):
        rows = slice(RC * c, RC * (c + 1))
        e = edge_input[rows].reshape(PC, D).astype(np.float64)
        e2 = e * e
        e2hi = e2.astype(ml_dtypes.bfloat16)
        e2lo = (e2 - e2hi.astype(np.float64)).astype(ml_dtypes.bfloat16)
        feat = np.zeros((NF, PC), ml_dtypes.bfloat16)
        feat[0, :] = 1.0
        feat[49, :] = 1.0
        feat[1:17, :] = e.T.astype(ml_dtypes.bfloat16)
        feat[17:33, :] = e2hi.T
        feat[33:49, :] = e2lo.T
        sp = spatial_pos[rows].reshape(PC).astype(np.int32)
        sp2 = NS * sp[0::2] + sp[1::2]             # combined pair-pair idx
        sw = sp2.reshape(NBLK, SCALLS, SJT // 32, 16).transpose(0, 3, 1, 2)
        sidx = np.ascontiguousarray(sw).reshape(128, SCALLS * SJT // 32)
        # rrep4[32a+k, s*TP+u] = 1/max(sp,1) of tile t=4s+a, col u
        r = (1.0 / np.maximum(sp, 1)).astype(np.float32).reshape(RC, N)
        rr = r.reshape(NGRP, GT, N).transpose(1, 0, 2)      # (4, 16, 512)
        rrep4 = np.ascontiguousarray(
            np.broadcast_to(rr[:, None, :, :], (GT, 32, NGRP, N))
        ).reshape(128, PC // 4)
        in_maps.append({
            "afeat": afeat, "mstat": mstat, "feat": feat,
            "tab_s": tab_s, "sidx": sidx.astype(np.int16), "rrep4": rrep4,
        })
    return in_maps


def _host_assemble(results):
    phi_spd = np.empty((H, N, N), np.float32)
    phi_edge = np.empty((H, N, N), np.float32)
    for c in range(NCORES):
        rs = slice(RC * c, RC * (c + 1))
        a = results[c]["ospd"].reshape(NBLK, 16, SCALLS, SJT // 2, 2, 2)
        phi_spd[:, rs, :] = a.transpose(1, 5, 0, 2, 3, 4).reshape(H, RC, N)
        b = results[c]["oedge"].reshape(GT, H, NGRP, N).astype(np.float32)
        phi_edge[:, rs, :] = b.transpose(1, 2, 0, 3).reshape(H, RC, N)
    return phi_spd, phi_edge


def kernel(spatial_pos, edge_input, max_dist, spd_table, edge_table,
           edge_dis_weight, _trace=False):
    spatial_pos = np.asarray(spatial_pos)
    edge_input = np.asarray(edge_input)
    spd_table = np.asarray(spd_table, dtype=np.float32)
    edge_table = np.asarray(edge_table, dtype=np.float32)
    edge_dis_weight = np.asarray(edge_dis_weight, dtype=np.float32)

    if "nc" not in _cached:
        _cached["nc"] = _build_nc()
    nc = _cached["nc"]

    in_maps = _host_prep(spatial_pos, edge_input, max_dist, spd_table,
                         edge_table, edge_dis_weight)
    res = run_bass_kernel_spmd(
        nc, in_maps, core_ids=list(range(NCORES)), trace=bool(_trace)
    )
    out = _host_assemble(res.results)
    if _trace:
        return out, res
    return out


# revision 6
# speedup vs baseline: 7.2381x; 7.2381x over previous
"""Trainium2 Bass kernel for BondEncoding2D (Graphormer-style bond encoding).

Computes, for a 512x512 node-pair grid:
  phi_spd[h,i,j]  = spd_table[spatial_pos[i,j], h]
  phi_edge[h,i,j] = (sum_d edge_table[edge_input[i,j,d]] @ W[d])[h] / max(spatial_pos[i,j],1)

Sharding: rows of the grid across 8 NeuronCores (64 rows / 32768 pairs each);
tables and weights replicated (per the sharding hint).

Per-core strategy (v2):
  * Host precomputes M[d] = edge_table @ W[d]; the edge term is then
    edge_sum[pair,:] = sum_d M[d, e_d(pair), :] — a 512-wide multi-hot
    (16 ones) contraction per pair.
  * The 512-wide multi-hot over c=(d,bond) is built exactly as
    relu(1 - (e_d - b)^2): a feature matmul (features [1, e_d, e_d^2-split]
    sent from host, exact in bf16) computes the integer argument on the PE
    (two concurrent 64-row bands), and the relu runs natively split across
    DVE and ACT producing exact 0/1 bf16.
  * Main matmuls: M (single bf16, chunk-major [128c' x 32k] per chunk) is
    the STATIONARY operand (32-col ldweights, cheap); the one-hot chunk is
    the MOVING operand (N=512/matmul), accumulating the 4 c-chunks into a
    [32, 512] PSUM strip. Tile t lands on column strip t%4, so 4 tiles
    share one [128, 512] PSUM bank and a single fused DVE epilogue
    multiplies by 1/denom (host-replicated) and converts to bf16.
  * phi_spd is a single exact-f32 GPSIMD ap_gather (runs concurrently with
    the PE pipeline).
  * Host reassembles the (32,512,512) outputs from the device layouts.
"""

import numpy as np
import ml_dtypes

import concourse.bass as bass
import concourse.bacc as bacc
import concourse.mybir as mybir
import concourse.tile as tile
from concourse.bass_utils import run_bass_kernel_spmd

N = 512          # atoms
D = 16           # max_dist
H = 32           # heads
NS = 64          # spatial values
NCORES = 8
RC = N // NCORES          # rows per core (64)
PC = RC * N               # pairs per core (32768)

TILES = 64                # PE tiles per core
TP = 512                  # pairs per tile (= one grid row)
NF = 64                   # feature rows (1 + 16*3 used, padded)
GT = 4                    # tiles per psum group (4 col strips)
NGRP = TILES // GT        # psum groups (16)
STAGE_G = 4               # groups per output DMA (16 tiles)
DVE_COLS = 840            # relu columns (of 1024 in ctA) done on DVE

# spd gather side
NBLK = 8                  # pair blocks (one per Q7 core)
SCALLS = 8                # gather calls
SJT = PC // NBLK // SCALLS  # pairs per block per call (512)

BF16 = mybir.dt.bfloat16
F32 = mybir.dt.float32
I16 = mybir.dt.int16

_cached = {}


def _build_nc(bench_reps=None, parts=("spd", "edge"), INTERLEAVE=True,
              NO_MAIN=False, NO_ARGS=False, FAKE_CT=False, EPI="mult",
              DVE_SPLIT=DVE_COLS):
    nc = bacc.Bacc(None, target_bir_lowering=False)

    afeat = nc.dram_tensor("afeat", [128, 512], BF16, kind="ExternalInput")
    mstat = nc.dram_tensor("mstat", [128, 128], BF16, kind="ExternalInput")
    feat = nc.dram_tensor("feat", [NF, PC], BF16, kind="ExternalInput")
    tab_s = nc.dram_tensor("tab_s", [128, 4096 * 4], F32, kind="ExternalInput")
    sidx = nc.dram_tensor("sidx", [128, SCALLS * SJT // 32], I16,
                          kind="ExternalInput")
    rrep4 = nc.dram_tensor("rrep4", [128, PC // 4], F32, kind="ExternalInput")
    oedge = nc.dram_tensor("oedge", [128, PC // 4], BF16,
                           kind="ExternalOutput")
    ospd = nc.dram_tensor("ospd", [128, SCALLS * SJT * 2], F32,
                          kind="ExternalOutput")

    mult = mybir.AluOpType.mult
    RELU = mybir.ActivationFunctionType.Relu

    with tile.TileContext(nc) as tc:
        with (
            tc.tile_pool(name="consts", bufs=1) as cpool,
            tc.tile_pool(name="arg_a", bufs=2, space="PSUM") as agpool_a,
            tc.tile_pool(name="arg_b", bufs=1, space="PSUM") as agpool_b,
            tc.tile_pool(name="outp", bufs=2, space="PSUM") as oppool,
            tc.tile_pool(name="ct", bufs=3) as ctpool,
            tc.tile_pool(name="stage", bufs=2) as stpool,
            tc.tile_pool(name="spd", bufs=2) as spool,
        ):
            afeat_t = cpool.tile([128, 512], BF16)
            nc.sync.dma_start(afeat_t[:], afeat[:])
            mstat_t = cpool.tile([128, 128], BF16)
            nc.sync.dma_start(mstat_t[:], mstat[:])
            tabs_t = cpool.tile([128, 4096 * 4], F32)
            nc.sync.dma_start(tabs_t[:], tab_s[:])
            rrep_t = cpool.tile([128, PC // 4], F32)
            nc.sync.dma_start(rrep_t[:], rrep4[:])
            featrep = cpool.tile([128, PC], BF16)
            for half in range(2):
                nc.sync.dma_start(featrep[64 * half:64 * half + NF, :], feat[:])

            import contextlib
            loop_cm = (
                tc.For_i(0, bench_reps, 1) if bench_reps
                else contextlib.nullcontext()
            )
            with loop_cm:
                # ---- phi_edge PE pipeline with spd gathers interleaved
                def spd_call(s):
                    si = spool.tile([128, SJT // 32], I16, tag="si")
                    nc.sync.dma_start(
                        si[:], sidx[:, s * (SJT // 32):(s + 1) * (SJT // 32)]
                    )
                    gs = spool.tile([128, SJT * 2], F32, tag="gs")
                    nc.gpsimd.ap_gather(
                        gs[:].rearrange("p (i v) -> p i v", v=4),
                        tabs_t[:].rearrange("p (n v) -> p n v", v=4),
                        si[:], channels=128, num_elems=4096, d=4,
                        num_idxs=SJT // 2,
                    )
                    nc.sync.dma_start(
                        ospd[:, s * SJT * 2:(s + 1) * SJT * 2], gs[:]
                    )

                if "edge" not in parts or not INTERLEAVE:
                    for s in range(SCALLS if "spd" in parts else 0):
                        spd_call(s)
                import collections
                pend = collections.deque()
                SKEW = 2
                ntl = TILES if "edge" in parts else 0
                op = None
                st = None
                for tt in range(ntl + SKEW):
                    if tt < ntl:
                        t = tt
                        if ("spd" in parts and INTERLEAVE
                                and t % (TILES // SCALLS) == 0):
                            spd_call(t // (TILES // SCALLS))
                        # one-hot args: arg[c,pair] = 1-(e_d(c)-b(c))^2 on PE
                        if FAKE_CT:
                            ctA = ctpool.tile([128, 2 * TP], BF16, tag="ctA")
                            ctB = ctpool.tile([128, 2 * TP], BF16, tag="ctB")
                            pend.append((t, ctA, ctB))
                        else:
                            args = []
                            for ab in range(2):  # two double-bank psum tiles
                                pool = agpool_a if ab == 0 else agpool_b
                                ag = pool.tile([128, 2 * TP], F32,
                                               tag=f"ag{ab}")
                                for qq in range(2):  # chunks q = 2*ab + qq
                                    q = 2 * ab + qq
                                    nc.tensor.matmul(
                                        ag[:, qq * TP:(qq + 1) * TP],
                                        afeat_t[64 * qq:64 * qq + 64,
                                                128 * q:128 * q + 128],
                                        featrep[64 * qq:64 * qq + 64,
                                                t * TP:(t + 1) * TP],
                                        start=True, stop=True,
                                        tile_position=(64 * qq, 0),
                                    )
                                args.append(ag)
                            if NO_ARGS is False:
                                # relu -> exact one-hot bf16 (DVE + ACT)
                                ctA = ctpool.tile([128, 2 * TP], BF16,
                                                  tag="ctA")
                                nc.vector.tensor_relu(
                                    ctA[:, :DVE_SPLIT], args[0][:, :DVE_SPLIT])
                                if DVE_SPLIT < 2 * TP:
                                    nc.scalar.activation(
                                        ctA[:, DVE_SPLIT:],
                                        args[0][:, DVE_SPLIT:], RELU)
                                ctB = ctpool.tile([128, 2 * TP], BF16,
                                                  tag="ctB")
                                nc.scalar.activation(ctB[:], args[1][:], RELU)
                                pend.append((t, ctA, ctB))
                    if tt < SKEW - 1 or not pend:
                        continue
                    if tt < ntl and len(pend) <= SKEW - 1:
                        continue
                    t, ctA, ctB = pend.popleft()
                    if NO_MAIN:
                        continue
                    # main matmuls: M stationary (32 cols), one-hot moving.
                    # tile t -> column strip a=t%4 of the group psum tile.
                    a = t % GT
                    if a == 0:
                        op = oppool.tile([128, TP], F32, tag="op")
                    for q in range(4):
                        ct = ctA if q < 2 else ctB
                        qq = q % 2
                        nc.tensor.matmul(
                            op[32 * a:32 * a + 32, :],
                            mstat_t[:, 32 * q:32 * q + 32],
                            ct[:, qq * TP:(qq + 1) * TP],
                            start=(q == 0), stop=(q == 3),
                            tile_position=(0, 32 * a),
                        )
                    if a == GT - 1:
                        # fused epilogue for the 4-tile group: x (1/denom),
                        # f32 PSUM -> bf16 staging SBUF
                        s = t // GT
                        sl = s % STAGE_G
                        if sl == 0:
                            st = stpool.tile([128, TP * STAGE_G], BF16,
                                             tag="st")
                        if EPI == "relu":
                            nc.vector.tensor_relu(
                                st[:, sl * TP:(sl + 1) * TP], op[:])
                        else:
                            nc.vector.tensor_tensor(
                                st[:, sl * TP:(sl + 1) * TP], op[:],
                                rrep_t[:, s * TP:(s + 1) * TP], mult)
                        if sl == STAGE_G - 1:
                            b = s // STAGE_G
                            nc.sync.dma_start(
                                oedge[:, b * TP * STAGE_G:
                                      (b + 1) * TP * STAGE_G],
                                st[:],
                            )
    nc.compile()
    return nc


def _host_prep(spatial_pos, edge_input, max_dist, spd_table, edge_table,
               edge_dis_weight):
    """Build per-core input maps (all numpy)."""
    md = int(max_dist)
    assert md == D
    W = edge_dis_weight.reshape(-1, H, H)[:md].astype(np.float64)
    M = edge_table.astype(np.float64) @ W          # (16, 32, 32)

    cp = np.arange(128)
    bb = (cp % 32).astype(np.float64)              # bond id per c'
    # feature-matmul weights: arg = (1-b^2)*1 + 2b*e_d - e2hi_d - e2lo_d
    afeat = np.zeros((128, 512), np.float64)
    for q in range(4):
        dsel = 4 * q + cp // 32                    # d(c') per column
        blk = np.zeros((64, 128))
        cc = 1.0 - bb * bb
        cchi = cc.astype(ml_dtypes.bfloat16).astype(np.float64)
        blk[0, :] = cchi
        blk[49, :] = cc - cchi
        blk[1 + dsel, cp] = 2.0 * bb
        blk[17 + dsel, cp] = -1.0
        blk[33 + dsel, cp] = -1.0
        afeat[64 * (q % 2):64 * (q % 2) + 64, 128 * q:128 * q + 128] = blk
    afeat = afeat.astype(ml_dtypes.bfloat16)

    # mstat[c', 32q+k] = bf16(M[4q + c'//32, c'%32, k])  (chunk-major)
    mstat = np.zeros((128, 128), ml_dtypes.bfloat16)
    for q in range(4):
        mstat[:, 32 * q:32 * q + 32] = M[4 * q + cp // 32, cp % 32, :]

    # spd pair-combined gather table: partition 16g+hh holds head cols
    # (2hh, 2hh+1) for both members of the pair-pair (sa, sb) = (c//64, c%64)
    hh = (np.arange(128) % 16)
    X = np.ascontiguousarray(
        spd_table[:, np.stack([2 * hh, 2 * hh + 1], 1)].transpose(1, 0, 2)
    ).astype(np.float32)                           # (128, 64, 2)
    T4 = np.empty((128, NS, NS, 4), np.float32)
    T4[:, :, :, 0:2] = X[:, :, None, :]
    T4[:, :, :, 2:4] = X[:, None, :, :]
    tab_s = T4.reshape(128, 4096 * 4)

    in_maps = []
    for c in range(NCORES):
        rows = slice(RC * c, RC * (c + 1))
        e = edge_input[rows].reshape(PC, D).astype(np.float64)
        e2 = e * e
        e2hi = e2.astype(ml_dtypes.bfloat16)
        e2lo = (e2 - e2hi.astype(np.float64)).astype(ml_dtypes.bfloat16)
        feat = np.zeros((NF, PC), ml_dtypes.bfloat16)
        feat[0, :] = 1.0
        feat[49, :] = 1.0
        feat[1:17, :] = e.T.astype(ml_dtypes.bfloat16)
        feat[17:33, :] = e2hi.T
        feat[33:49, :] = e2lo.T
        sp = spatial_pos[rows].reshape(PC).astype(np.int32)
        sp2 = NS * sp[0::2] + sp[1::2]             # combined pair-pair idx
        sw = sp2.reshape(NBLK, SCALLS, SJT // 32, 16).transpose(0, 3, 1, 2)
        sidx = np.ascontiguousarray(sw).reshape(128, SCALLS * SJT // 32)
        # rrep4[32a+k, s*TP+u] = 1/max(sp,1) of tile t=4s+a, col u
        r = (1.0 / np.maximum(sp, 1)).astype(np.float32).reshape(RC, N)
        rr = r.reshape(NGRP, GT, N).transpose(1, 0, 2)      # (4, 16, 512)
        rrep4 = np.ascontiguousarray(
            np.broadcast_to(rr[:, None, :, :], (GT, 32, NGRP, N))
        ).reshape(128, PC // 4)
        in_maps.append({
            "afeat": afeat, "mstat": mstat, "feat": feat,
            "tab_s": tab_s, "sidx": sidx.astype(np.int16), "rrep4": rrep4,
        })
    return in_maps


def _host_assemble(results):
    phi_spd = np.empty((H, N, N), np.float32)
    phi_edge = np.empty((H, N, N), np.float32)
    for c in range(NCORES):
        rs = slice(RC * c, RC * (c + 1))
        a = results[c]["ospd"].reshape(NBLK, 16, SCALLS, SJT // 2, 2, 2)
        phi_spd[:, rs, :] = a.transpose(1, 5, 0, 2, 3, 4).reshape(H, RC, N)
        b = results[c]["oedge"].reshape(GT, H, NGRP, N).astype(np.float32)
        phi_edge[:, rs, :] = b.transpose(1, 2, 0, 3).reshape(H, RC, N)
    return phi_spd, phi_edge


def kernel(spatial_pos, edge_input, max_dist, spd_table, edge_table,
           edge_dis_weight, _trace=False):
    spatial_pos = np.asarray(spatial_pos)
    edge_input = np.asarray(edge_input)
    spd_table = np.asarray(spd_table, dtype=np.float32)
    edge_table = np.asarray(edge_table, dtype=np.float32)
    edge_dis_weight = np.asarray(edge_dis_weight, dtype=np.float32)

    if "nc" not in _cached:
        _cached["nc"] = _build_nc()
    nc = _cached["nc"]

    in_maps = _host_prep(spatial_pos, edge_input, max_dist, spd_table,
                         edge_table, edge_dis_weight)
    res = run_bass_kernel_spmd(
        nc, in_maps, core_ids=list(range(NCORES)), trace=bool(_trace)
    )
    out = _host_assemble(res.results)
    if _trace:
        return out, res
    return out


# revision 8
# speedup vs baseline: 11.9065x; 1.6450x over previous
"""Trainium2 Bass kernel for BondEncoding2D (Graphormer-style bond encoding).

Computes, for a 512x512 node-pair grid:
  phi_spd[h,i,j]  = spd_table[spatial_pos[i,j], h]
  phi_edge[h,i,j] = (sum_d edge_table[edge_input[i,j,d]] @ W[d])[h] / max(spatial_pos[i,j],1)

Sharding: rows of the grid across 8 NeuronCores (64 rows / 32768 pairs each);
tables and weights replicated (per the sharding hint).

Per-core strategy (v3 — everything on the PE one-hot pipeline):
  * Host precomputes M[d] = edge_table @ W[d]; the edge term is
    edge_sum[pair,:] = sum_d M[d, e_d(pair), :] — a 512-wide multi-hot
    (16 ones) contraction per pair.  phi_spd is a 64-wide one-hot gather.
  * One-hots are built exactly as relu(integer quadratic): feature matmuls
    on the PE (exact in bf16) compute the args; relu on DVE/ACT produces
    exact 0/1 bf16.  The spd arg matmul (K=54 on row strips 2-3) runs
    concurrently with the first edge arg matmul (K=50 on strips 0-1).
  * Main matmuls: tables are the STATIONARY operand (32-col ldweights,
    cheap); one-hots are MOVING (N=512).  Each [128, 512] PSUM bank holds
    a 2-tile group: partitions 0-63 = edge_sum of tiles (2g, 2g+1),
    partitions 64-127 = phi_spd of the same tiles.  Epilogue: one DVE
    tensor_tensor multiplies the edge half by 1/denom (host-replicated) and
    one ACT copy moves the spd half; both convert f32 PSUM -> bf16 SBUF.
  * GPSIMD is unused (ap_gather measured ~183 us — too slow).
  * Host reassembles the (32,512,512) outputs from the device layout.
"""

import numpy as np
import ml_dtypes

import concourse.bass as bass
import concourse.bacc as bacc
import concourse.mybir as mybir
import concourse.tile as tile
from concourse.bass_utils import run_bass_kernel_spmd

N = 512          # atoms
D = 16           # max_dist
H = 32           # heads
NS = 64          # spatial values
NCORES = 8
RC = N // NCORES          # rows per core (64)
PC = RC * N               # pairs per core (32768)

TILES = 64                # PE tiles per core
TP = 512                  # pairs per tile (= one grid row)
NF = 64                   # feature rows (50 edge + 4 spd)
GT = 2                    # tiles per psum group
NGRP = TILES // GT        # psum groups (32)
STAGE_G = 4               # groups per output DMA (8 tiles)
DVE_COLS = 1024           # relu columns (of 2048 ct cols/tile) done on DVE

BF16 = mybir.dt.bfloat16
F32 = mybir.dt.float32

_cached = {}


def _build_nc(bench_reps=None, parts=("spd", "edge"), DVE_SPLIT=DVE_COLS):
    nc = bacc.Bacc(None, target_bir_lowering=False)

    afeat = nc.dram_tensor("afeat", [128, 512], BF16, kind="ExternalInput")
    afs = nc.dram_tensor("afs", [128, 64], BF16, kind="ExternalInput")
    mstat = nc.dram_tensor("mstat", [128, 128], BF16, kind="ExternalInput")
    stab = nc.dram_tensor("stab", [64, 32], BF16, kind="ExternalInput")
    feat = nc.dram_tensor("feat", [NF, PC], BF16, kind="ExternalInput")
    rrep2 = nc.dram_tensor("rrep2", [64, PC // 2], F32, kind="ExternalInput")
    oboth = nc.dram_tensor("oboth", [128, PC // 2], BF16,
                           kind="ExternalOutput")

    mult = mybir.AluOpType.mult
    RELU = mybir.ActivationFunctionType.Relu
    COPY = mybir.ActivationFunctionType.Copy

    with tile.TileContext(nc) as tc:
        with (
            tc.tile_pool(name="consts", bufs=1) as cpool,
            tc.tile_pool(name="arg_a", bufs=1, space="PSUM") as agpool_a,
            tc.tile_pool(name="arg_b", bufs=1, space="PSUM") as agpool_b,
            tc.tile_pool(name="arg_s", bufs=2, space="PSUM") as agpool_s,
            tc.tile_pool(name="outp", bufs=2, space="PSUM") as oppool,
            tc.tile_pool(name="ct", bufs=3) as ctpool,
            tc.tile_pool(name="cts", bufs=3) as ctspool,
            tc.tile_pool(name="stage", bufs=2) as stpool,
        ):
            afeat_t = cpool.tile([128, 512], BF16)
            nc.sync.dma_start(afeat_t[:], afeat[:])
            afs_t = cpool.tile([128, 64], BF16)
            nc.sync.dma_start(afs_t[:], afs[:])
            mstat_t = cpool.tile([128, 128], BF16)
            nc.sync.dma_start(mstat_t[:], mstat[:])
            stab_t = cpool.tile([64, 32], BF16)
            nc.sync.dma_start(stab_t[:], stab[:])
            rrep_t = cpool.tile([64, PC // 2], F32)
            nc.sync.dma_start(rrep_t[:], rrep2[:])
            featrep = cpool.tile([128, PC], BF16)
            for half in range(2):
                nc.sync.dma_start(featrep[64 * half:64 * half + NF, :], feat[:])

            import contextlib
            loop_cm = (
                tc.For_i(0, bench_reps, 1) if bench_reps
                else contextlib.nullcontext()
            )
            with loop_cm:
                import collections
                pend = collections.deque()
                SKEW = 2
                op = None
                st = None
                for tt in range(TILES + SKEW):
                    if tt < TILES:
                        t = tt
                        # edge one-hot args: arg[c,p] = 1-(e_d(c)-b(c))^2.
                        # chunks 0,1 on row strips {0,1}+{2,3}; spd args
                        # (K=54, strips {2,3}) run while chunk 0 streams.
                        ags = agpool_s.tile([64, TP], F32, tag="ags")
                        nc.tensor.matmul(
                            ags[:],
                            afs_t[64:64 + 54, :],
                            featrep[64:64 + 54, t * TP:(t + 1) * TP],
                            start=True, stop=True,
                            tile_position=(64, 0),
                        )
                        args = []
                        for ab in range(2):   # two double-bank psum tiles
                            pool = agpool_a if ab == 0 else agpool_b
                            ag = pool.tile([128, 2 * TP], F32, tag=f"ag{ab}")
                            for qq in range(2):   # chunks q = 2*ab + qq
                                q = 2 * ab + qq
                                nc.tensor.matmul(
                                    ag[:, qq * TP:(qq + 1) * TP],
                                    afeat_t[64 * qq:64 * qq + 64,
                                            128 * q:128 * q + 128],
                                    featrep[64 * qq:64 * qq + 64,
                                            t * TP:(t + 1) * TP],
                                    start=True, stop=True,
                                    tile_position=(64 * qq, 0),
                                )
                            args.append(ag)
                        # relu -> exact one-hot bf16, split DVE/ACT
                        cts = ctspool.tile([64, TP], BF16, tag="cts")
                        nc.scalar.activation(cts[:], ags[:], RELU)
                        ctA = ctpool.tile([128, 2 * TP], BF16, tag="ctA")
                        nc.vector.tensor_relu(
                            ctA[:, :DVE_SPLIT], args[0][:, :DVE_SPLIT])
                        if DVE_SPLIT < 2 * TP:
                            nc.scalar.activation(
                                ctA[:, DVE_SPLIT:], args[0][:, DVE_SPLIT:],
                                RELU)
                        ctB = ctpool.tile([128, 2 * TP], BF16, tag="ctB")
                        nc.scalar.activation(ctB[:], args[1][:], RELU)
                        pend.append((t, ctA, ctB, cts))
                    if tt < SKEW - 1 or not pend:
                        continue
                    if tt < TILES and len(pend) <= SKEW - 1:
                        continue
                    t, ctA, ctB, cts = pend.popleft()
                    # main matmuls: tables stationary (32 cols), one-hot
                    # moving.  2-tile psum groups: partitions 32a hold
                    # a=0: edge(2g), a=1: edge(2g+1), a=2: spd(2g),
                    # a=3: spd(2g+1).
                    a = t % GT
                    if a == 0:
                        op = oppool.tile([128, TP], F32, tag="op")
                    for q in range(4):
                        ct = ctA if q < 2 else ctB
                        qq = q % 2
                        nc.tensor.matmul(
                            op[32 * a:32 * a + 32, :],
                            mstat_t[:, 32 * q:32 * q + 32],
                            ct[:, qq * TP:(qq + 1) * TP],
                            start=(q == 0), stop=(q == 3),
                            tile_position=(0, 32 * a),
                        )
                    nc.tensor.matmul(
                        op[64 + 32 * a:64 + 32 * a + 32, :],
                        stab_t[:],
                        cts[:],
                        start=True, stop=True,
                        tile_position=(0, 64 + 32 * a),
                    )
                    if a == GT - 1:
                        # epilogue per 2-tile group: DVE multiplies the edge
                        # half by 1/denom; ACT copies the spd half.
                        g = t // GT
                        sl = g % STAGE_G
                        if sl == 0:
                            st = stpool.tile([128, TP * STAGE_G], BF16,
                                             tag="st")
                        nc.vector.tensor_tensor(
                            st[:64, sl * TP:(sl + 1) * TP], op[:64, :],
                            rrep_t[:, g * TP:(g + 1) * TP], mult)
                        nc.scalar.activation(
                            st[64:, sl * TP:(sl + 1) * TP], op[64:, :], COPY)
                        if sl == STAGE_G - 1:
                            b = g // STAGE_G
                            nc.sync.dma_start(
                                oboth[:, b * TP * STAGE_G:
                                      (b + 1) * TP * STAGE_G],
                                st[:],
                            )
    nc.compile()
    return nc


def _host_prep(spatial_pos, edge_input, max_dist, spd_table, edge_table,
               edge_dis_weight):
    """Build per-core input maps (all numpy)."""
    md = int(max_dist)
    assert md == D
    W = edge_dis_weight.reshape(-1, H, H)[:md].astype(np.float64)
    M = edge_table.astype(np.float64) @ W          # (16, 32, 32)

    cp = np.arange(128)
    bb = (cp % 32).astype(np.float64)              # bond id per c'
    # feature-matmul weights: arg = (1-b^2)*1 + 2b*e_d - e2hi_d - e2lo_d
    afeat = np.zeros((128, 512), np.float64)
    for q in range(4):
        dsel = 4 * q + cp // 32                    # d(c') per column
        blk = np.zeros((64, 128))
        cc = 1.0 - bb * bb
        cchi = cc.astype(ml_dtypes.bfloat16).astype(np.float64)
        blk[0, :] = cchi
        blk[49, :] = cc - cchi
        blk[1 + dsel, cp] = 2.0 * bb
        blk[17 + dsel, cp] = -1.0
        blk[33 + dsel, cp] = -1.0
        afeat[64 * (q % 2):64 * (q % 2) + 64, 128 * q:128 * q + 128] = blk
    afeat = afeat.astype(ml_dtypes.bfloat16)

    # spd arg weights: arg_s[c,p] = (1-uc^2-vc^2) + 2uc*u - u^2 + 2vc*v - v^2
    # with sp = 8u + v; features at rows {0 (ones), 50:u, 51:u^2, 52:v,
    # 53:v^2}; all integers exact in bf16.
    cs = np.arange(64)
    uc, vc = cs // 8, cs % 8
    afs = np.zeros((128, 64))
    afs[64 + 0, :] = 1.0 - uc * uc - vc * vc
    afs[64 + 50, :] = 2.0 * uc
    afs[64 + 51, :] = -1.0
    afs[64 + 52, :] = 2.0 * vc
    afs[64 + 53, :] = -1.0
    afs = afs.astype(ml_dtypes.bfloat16)

    # mstat[c', 32q+k] = bf16(M[4q + c'//32, c'%32, k])  (chunk-major)
    mstat = np.zeros((128, 128), ml_dtypes.bfloat16)
    for q in range(4):
        mstat[:, 32 * q:32 * q + 32] = M[4 * q + cp // 32, cp % 32, :]

    stab = spd_table.astype(ml_dtypes.bfloat16)    # (64, 32)

    in_maps = []
    for c in range(NCORES):
        rows = slice(RC * c, RC * (c + 1))
        e = edge_input[rows].reshape(PC, D).astype(np.float64)
        e2 = e * e
        e2hi = e2.astype(ml_dtypes.bfloat16)
        e2lo = (e2 - e2hi.astype(np.float64)).astype(ml_dtypes.bfloat16)
        feat = np.zeros((NF, PC), ml_dtypes.bfloat16)
        feat[0, :] = 1.0
        feat[49, :] = 1.0
        feat[1:17, :] = e.T.astype(ml_dtypes.bfloat16)
        feat[17:33, :] = e2hi.T
        feat[33:49, :] = e2lo.T
        sp = spatial_pos[rows].reshape(PC).astype(np.int32)
        feat[50, :] = (sp // 8).astype(np.float64)
        feat[51, :] = ((sp // 8) ** 2).astype(np.float64)
        feat[52, :] = (sp % 8).astype(np.float64)
        feat[53, :] = ((sp % 8) ** 2).astype(np.float64)
        # rrep2[32a+k, g*TP+u] = 1/max(sp,1) of tile t=2g+a, col u  (a=0,1)
        r = (1.0 / np.maximum(sp, 1)).astype(np.float32).reshape(RC, N)
        rr = r.reshape(NGRP, GT, N).transpose(1, 0, 2)      # (2, 32, 512)
        rrep2 = np.ascontiguousarray(
            np.broadcast_to(rr[:, None, :, :], (GT, 32, NGRP, N))
        ).reshape(64, PC // 2)
        in_maps.append({
            "afeat": afeat, "afs": afs, "mstat": mstat, "stab": stab,
            "feat": feat, "rrep2": rrep2,
        })
    return in_maps


def _host_assemble(results):
    phi_spd = np.empty((H, N, N), np.float32)
    phi_edge = np.empty((H, N, N), np.float32)
    for c in range(NCORES):
        rs = slice(RC * c, RC * (c + 1))
        b = results[c]["oboth"].reshape(4, H, NGRP, N).astype(np.float32)
        phi_edge[:, rs, :] = b[0:2].transpose(1, 2, 0, 3).reshape(H, RC, N)
        phi_spd[:, rs, :] = b[2:4].transpose(1, 2, 0, 3).reshape(H, RC, N)
    return phi_spd, phi_edge


def kernel(spatial_pos, edge_input, max_dist, spd_table, edge_table,
           edge_dis_weight, _trace=False):
    spatial_pos = np.asarray(spatial_pos)
    edge_input = np.asarray(edge_input)
    spd_table = np.asarray(spd_table, dtype=np.float32)
    edge_table = np.asarray(edge_table, dtype=np.float32)
    edge_dis_weight = np.asarray(edge_dis_weight, dtype=np.float32)

    if "nc" not in _cached:
        _cached["nc"] = _build_nc()
    nc = _cached["nc"]

    in_maps = _host_prep(spatial_pos, edge_input, max_dist, spd_table,
                         edge_table, edge_dis_weight)
    res = run_bass_kernel_spmd(
        nc, in_maps, core_ids=list(range(NCORES)), trace=bool(_trace)
    )
    out = _host_assemble(res.results)
    if _trace:
        return out, res
    return out
